# revision 6
# baseline (speedup 1.0000x reference)
"""Trainium2 Bass kernel for nn_Mixture_Loss_74053826118054.

Strategy (pure data parallel: batch axis B=256 sharded over 8 cores):
  Every term of the loss depends only on 5 per-(s,b)-row reductions over D:
    ll = sum_d l^2,  tt = sum_d t^2,  lt = sum_d l*t,
    ln = sum_d l[s]*l[s+1]  (consecutive sentences, same batch),
    tn = sum_d t[s]*t[s+1]
  Each core computes those row arrays for its 32 batches; the tiny O(S*B)
  finish (cos, deltas, rank-compaction, delta-of-delta) runs on host in
  float64, reproducing the reference semantics exactly.

Numerics: ln/tn feed the delta-of-delta term, which divides by near-zero
deltas — they need fp32 inputs + fp32 accumulation (absolute-scale noise
on the cosine blows up 5+ orders of magnitude). lt only enters smooth
averaged terms; computing it from fp16-rounded inputs moves the final
result by < 1e-5 (verified on the reference data).

Engine assignment (v3), driven by HW traces of v1/v2:
  - DVE fused scalar_tensor_tensor is ~1221 ns/[128,1024] when GpSimd is
    idle but ~3352 ns under GpSimd SBUF traffic -> GpSimd runs no compute.
  - ACT: ll, tt squares with fused accumulate (32 ops, contention-immune).
  - DVE: ln, tn as fused stt product+accumulate (32 ops).
  - PE (else idle): lt as fp16 Gram-diagonal blocks. The host stages
    d-major fp16 copies of l and t ([d-chunk partition, row] layout);
    for each 128-row block, 8 accumulating [128x128x128] fp16 matmuls
    produce G = HL^T HT in PSUM whose diagonal is lt for those rows.
    DVE extracts each diagonal with one [128,128] stt against an identity
    matrix (G (*) I, accumulate) at ~0.4 us/block.
Row-major fp32 chunks stream on the Sync engine's DMA ring; the d-major
fp16 quarters stream in parallel on GpSimd's software-DGE ring into a
3-buffer SBUF ring (quarter 3 reuses buffer 0 once PE finished quarter 0).
"""

import numpy as np

from contextlib import ExitStack

import concourse.bass as bass
import concourse.mybir as mybir
from concourse.bass_utils import run_bass_kernel_spmd

F32 = mybir.dt.float32
F16 = mybir.dt.float16
AF = mybir.ActivationFunctionType
ALU = mybir.AluOpType

N_CORES = 8
S, B, D = 64, 256, 1024
B_SHARD = B // N_CORES          # 32 batches per core
ROWS = B_SHARD * S              # 2048 real rows per core
G = 16                          # rows per window
P = 128                         # partitions per tile
ROWS_PAD = (P + 1) * G          # one extra window of padding rows
NCOL = G                        # 16 result columns per quantity
QUANTS = ("ll", "tt", "lt", "ln", "tn")
NQ = 4                          # row-quarters (d-major streaming granularity)
QR = ROWS // NQ                 # 512 rows per quarter
NBLK = ROWS // P                # 16 Gram blocks of 128 rows
DC = D // P                     # 8 d-chunks
XTBUFS = 3                      # d-major SBUF ring depth

_cached_nc = None


def _build_program():
    global _cached_nc
    if _cached_nc is not None:
        return _cached_nc
    nc = bass.Bass()
    x_in = nc.dram_tensor("x", [2, ROWS_PAD, D], F32, kind="ExternalInput")
    xt_in = nc.dram_tensor("xt", [NQ, P, 2, DC, QR], F16,
                           kind="ExternalInput")
    eye_in = nc.dram_tensor("eye", [P, P], F32, kind="ExternalInput")
    res_out = nc.dram_tensor("res", [P, 5 * NCOL], F32, kind="ExternalOutput")
    x_v = x_in.rearrange("h (w g) d -> w h g d", g=G)

    with ExitStack() as stack:
        ec = stack.enter_context
        csem = [ec(nc.semaphore(f"c{j}")) for j in range(G + 1)]
        xbig = ec(nc.sbuf_tensor([P, (G + 1) * 2 * D], F32))
        xtbuf = ec(nc.sbuf_tensor([P, XTBUFS * 2 * DC * QR], F16))
        eye = ec(nc.sbuf_tensor([P, P], F32))
        dummies = ec(nc.sbuf_tensor([P, 8], F32))
        res = ec(nc.sbuf_tensor([P, 5 * NCOL], F32))
        psum = [ec(nc.psum_tensor(f"psum{i}", [P, P], F32))
                for i in range(4)]
        xt_sem = ec(nc.semaphore("xt_sem"))
        pe_sem = ec(nc.semaphore("pe_sem"))
        ext_sem = ec(nc.semaphore("ext_sem"))
        done_sem = ec(nc.semaphore("done_sem"))
        out_sem = ec(nc.semaphore("out_sem"))
        block = ec(nc.Block())
        roff = {q: i * NCOL for i, q in enumerate(QUANTS)}
        xc = xbig.ap().rearrange("p (c v d) -> p c v d", v=2, d=D)
        xtv = xtbuf.ap().rearrange("p (b h c r) -> p b h c r", b=XTBUFS,
                                   h=2, r=QR)

        def chunk(j, half):
            return xc[:, j, half, :]

        def rcol(q, j):
            k = roff[q] + j
            return res.ap()[:, k:k + 1]

        def bcast(k, n=D):
            return dummies.ap()[:, k:k + 1].broadcast_to((P, n))

        @block.sync
        def _(sync):
            sync.dma_start(out=eye.ap(), in_=eye_in[:, :]).then_inc(
                ext_sem, 16)  # reuse ext_sem for the eye load (-16 offset)
            for j in range(G + 1):
                src = x_v[0:P, :, j, :] if j < G else x_v[1:P + 1, :, 0, :]
                sync.dma_start(out=xc[:, j, :, :], in_=src).then_inc(
                    csem[j], 16)
            sync.wait_ge(done_sem, 2)
            sync.dma_start(out=res_out[:, :], in_=res.ap()).then_inc(
                out_sem, 16)
            sync.wait_ge(out_sem, 16)

        @block.gpsimd
        def _(gpsimd):
            # d-major fp16 quarters on the software-DGE ring, 3-buffer ring
            for q in range(XTBUFS):
                gpsimd.dma_start(out=xtv[:, q, :, :, :],
                                 in_=xt_in[q]).then_inc(xt_sem, 16)
            gpsimd.wait_ge(pe_sem, NQ)      # PE done reading quarter 0
            gpsimd.dma_start(out=xtv[:, 0, :, :, :],
                             in_=xt_in[XTBUFS]).then_inc(xt_sem, 16)

        @block.scalar
        def _(scalar):
            for j in range(G):
                scalar.wait_ge(csem[j], 16)
                ins = scalar.activation(bcast(0), chunk(j, 0), AF.Square,
                                        accum_out=rcol("ll", j))
                ins = scalar.activation(bcast(1), chunk(j, 1), AF.Square,
                                        accum_out=rcol("tt", j))
            ins.then_inc(done_sem, 1)

        @block.tensor
        def _(tensor):
            for k in range(NBLK):
                q = k // NQ
                m = k % NQ
                tensor.wait_ge(xt_sem, 16 * (q + 1))
                if k >= 4:
                    # +16: eye-load increments ride on ext_sem
                    tensor.wait_ge(ext_sem, 16 + (k - 3))
                for c in range(DC):
                    ins = tensor.matmul(
                        out=psum[k % 4].ap(),
                        lhsT=xtv[:, q % XTBUFS, 0, c, m * P:(m + 1) * P],
                        rhs=xtv[:, q % XTBUFS, 1, c, m * P:(m + 1) * P],
                        start=(c == 0), stop=(c == DC - 1))
                ins.then_inc(pe_sem, 1)

        @block.vector
        def _(vector):
            for j in range(G):
                if j == 0:
                    vector.wait_ge(csem[0], 16)
                    vector.wait_ge(ext_sem, 16)   # eye loaded
                vector.wait_ge(csem[j + 1], 16)
                vector.scalar_tensor_tensor(
                    out=bcast(2), in0=chunk(j, 0), scalar=0.0,
                    in1=chunk(j + 1, 0), op0=ALU.bypass, op1=ALU.mult,
                    accum_out=rcol("ln", j))
                ins = vector.scalar_tensor_tensor(
                    out=bcast(3), in0=chunk(j, 1), scalar=0.0,
                    in1=chunk(j + 1, 1), op0=ALU.bypass, op1=ALU.mult,
                    accum_out=rcol("tn", j))
                if j >= 8:
                    for k in (2 * (j - 8), 2 * (j - 8) + 1):
                        vector.wait_ge(pe_sem, k + 1)
                        ins = vector.scalar_tensor_tensor(
                            out=bcast(4, P), in0=psum[k % 4].ap(),
                            scalar=0.0, in1=eye.ap(), op0=ALU.bypass,
                            op1=ALU.mult, accum_out=rcol("lt", k))
                        if k < NBLK - 4:
                            # PE block k+4 waits on this; later blocks
                            # have no waiter (one sync update max/instr)
                            ins.then_inc(ext_sem, 1)
            ins.then_inc(done_sem, 1)

    _cached_nc = nc
    return nc


def _run_device(logits, tgt_out, trace=False):
    """Returns dict q -> (B, S) float32 row-dot arrays, plus kernel results."""
    nc = _build_program()
    # (S, B, D) -> (B, S, D) batch-major, split over cores along B
    lb = np.ascontiguousarray(np.swapaxes(logits, 0, 1))
    tb = np.ascontiguousarray(np.swapaxes(tgt_out, 0, 1))
    eye = np.eye(P, dtype=np.float32)
    in_maps = []
    for c in range(N_CORES):
        sl = slice(c * B_SHARD, (c + 1) * B_SHARD)
        lbs = lb[sl].reshape(ROWS, D)
        tbs = tb[sl].reshape(ROWS, D)
        x = np.zeros((2, ROWS_PAD, D), np.float32)
        x[0, :ROWS] = lbs
        x[1, :ROWS] = tbs
        # d-major fp16: xt[q, p, h, c, r] = X_h[QR*q + r, P*c + p]
        xt = np.empty((NQ, P, 2, DC, QR), np.float16)
        xt[:, :, 0] = lbs.astype(np.float16).reshape(
            NQ, QR, DC, P).transpose(0, 3, 2, 1)
        xt[:, :, 1] = tbs.astype(np.float16).reshape(
            NQ, QR, DC, P).transpose(0, 3, 2, 1)
        in_maps.append({"x": x, "xt": xt, "eye": eye})
    kres = run_bass_kernel_spmd(nc, in_maps, list(range(N_CORES)), trace=trace)
    full = {}
    for i, q in enumerate(QUANTS):
        parts = []
        for c in range(N_CORES):
            arr = kres.results[c]["res"][:, i * NCOL:(i + 1) * NCOL]
            if q == "lt":
                # column k = Gram block k: row r = P*k + partition
                flat = arr.T.reshape(ROWS)
            else:
                # column j = window slot j: row r = G*partition + j
                flat = arr.reshape(ROWS)
            parts.append(flat.reshape(B_SHARD, S))
        full[q] = np.concatenate(parts, axis=0)
    return full, kres


def _finish_host(rows, mask):
    """Host-side float64 finish: reproduce reference semantics exactly."""
    ll = rows["ll"].astype(np.float64)
    tt = rows["tt"].astype(np.float64)
    lt = rows["lt"].astype(np.float64)
    ln = rows["ln"].astype(np.float64)
    tn = rows["tn"].astype(np.float64)

    valid = ~mask                     # (B, S)
    n_valid = float(valid.sum())

    # masked MSE: sum over valid rows of sum_d (l-t)^2 = ll - 2lt + tt
    mse = ((ll - 2.0 * lt + tt) * valid).sum() / (n_valid * D)

    # CosineEmbeddingLoss part (eps = 1e-8)
    na = np.maximum(np.sqrt(ll), 1e-8)
    nb = np.maximum(np.sqrt(tt), 1e-8)
    c = lt / (na * nb)
    loss_cos = ((1.0 - c) * valid).sum() / n_valid

    # consecutive-sentence cosine deltas (eps = 1e-6), shape (B, S-1)
    nl = np.maximum(np.sqrt(ll), 1e-6)
    nt = np.maximum(np.sqrt(tt), 1e-6)
    d_l = ln[:, :S - 1] / (nl[:, :-1] * nl[:, 1:])
    d_t = tn[:, :S - 1] / (nt[:, :-1] * nt[:, 1:])
    pair_valid = valid[:, :-1] & valid[:, 1:]
    cnt = int(pair_valid.sum())
    loss_delta = (np.square(d_l - d_t) * pair_valid).sum() / max(cnt, 1)

    # delta-of-delta on the compacted (valid-only, batch-major) delta lists
    L = B * (S - 1)
    pvf = pair_valid.reshape(-1)

    def dd(d_flat):
        dense = np.zeros(L, np.float64)
        dense[:cnt] = d_flat[pvf]
        prev = dense[:-1]
        den = np.where(prev != 0, prev, 1e-6)
        return (dense[1:] - prev) / den

    dd_l = dd(d_l.reshape(-1))
    dd_t = dd(d_t.reshape(-1))
    dd_valid = np.arange(L - 1) < (cnt - 1)
    n_dd = float(max(cnt - 1, 1))
    loss_dd = (np.square(dd_l - dd_t) * dd_valid).sum() / n_dd / 100.0

    return mse + loss_cos + loss_delta + loss_dd


def kernel(logits, tgt_out, tgt_padding_mask, _trace=False):
    logits = np.asarray(logits, dtype=np.float32)
    tgt_out = np.asarray(tgt_out, dtype=np.float32)
    mask = np.asarray(tgt_padding_mask).astype(bool)
    rows, kres = _run_device(logits, tgt_out, trace=_trace)
    total = _finish_host(rows, mask)
    out = np.array(total, dtype=np.float32)
    if _trace:
        return out, kres
    return out


# revision 12
# speedup vs baseline: 1.2988x; 1.2988x over previous
"""Trainium2 Bass kernel for nn_Mixture_Loss_74053826118054.

Strategy (pure data parallel: batch axis B=256 sharded over 8 cores):
  Every term of the loss depends only on 5 per-(s,b)-row reductions over D:
    ll = sum_d l^2,  tt = sum_d t^2,  lt = sum_d l*t,
    ln = sum_d l[s]*l[s+1]  (consecutive sentences, same batch),
    tn = sum_d t[s]*t[s+1]
  Each core computes those row arrays for its 32 batches; the tiny O(S*B)
  finish (cos, deltas, rank-compaction, delta-of-delta) runs on host in
  float64, reproducing the reference semantics exactly.

Device layout: rows are batch-major (b, s). Each SBUF partition holds a
window of 17 consecutive rows (16 + 1 overlap), so consecutive-row products
are free-axis slices (partition shifts are illegal on compute engines).
l and t are stacked into one DRAM tensor and each 1024-wide chunk (row slot
j of all 128 windows, both halves) is fetched with a single strided DMA.

Engine assignment (v2): profiling the v1 kernel showed DVE's fused
scalar_tensor_tensor runs at ~1213 ns when GpSimd is idle but ~3352 ns
while GpSimd tensor_tensor traffic hits SBUF (2.8x port contention), while
ACT activations are contention-immune at 1131+278 ns. So v2 bans GpSimd:
  ACT: ll, tt squares with fused accumulate        (32 ops, ~45 us busy)
  DVE: ln, tn, lt as fused stt product+accumulate  (48 ops, ~62 us busy)
DMA (17.4 MB/core) streams underneath at ~46 us. No drains: the final
compute op of each engine carries the done increment (sem updates fire
after the read-accumulator aux op per the HW model).

v2.1 schedule refinements (from the v2 trace):
  - The first stt could only start at 18.8 us (DMA boot + issue + first two
    1 MB chunks). Chunks 0/1 are now fetched as d-halves (x0a,x1a,x0b,x1b)
    and chunk 0's reductions run as half-ops with separate accumulators
    (summed on host), so DVE starts ~6 us earlier.
  - The overlap chunk (16) is issued 6th instead of 17th: as the 17th ring
    entry it was observed to deliver 27 us after its predecessors.
  - Result columns are interleaved per chunk ([5 quants] x 16 chunks + 5
    half-spares) so the output can be shipped as two contiguous DMAs: cols
    0:70 as soon as both engines pass chunk 13, the rest at the end.
"""

import numpy as np

from contextlib import ExitStack

import concourse.bass as bass
import concourse.mybir as mybir
from concourse.bass_utils import run_bass_kernel_spmd

F32 = mybir.dt.float32
AF = mybir.ActivationFunctionType
ALU = mybir.AluOpType

N_CORES = 8
S, B, D = 64, 256, 1024
B_SHARD = B // N_CORES          # 32 batches per core
ROWS = B_SHARD * S              # 2048 real rows per core
G = 16                          # rows per window
P = 128                         # partitions per tile
NMEGA = ROWS // (G * P)         # 1 window-set per core
ROWS_PAD = (P * NMEGA + 1) * G  # one extra window of padding rows
NCOL = NMEGA * G                # 16 result columns
QUANTS = ("ll", "tt", "lt", "ln", "tn")

_cached_nc = None


def _build_program():
    global _cached_nc
    if _cached_nc is not None:
        return _cached_nc
    nc = bass.Bass()
    x_in = nc.dram_tensor("x", [2, ROWS_PAD, D], F32, kind="ExternalInput")
    res_out = nc.dram_tensor("res", [P, 5 * NCOL + 5], F32,
                             kind="ExternalOutput")
    x_v = x_in.rearrange("h (w g) d -> w h g d", g=G)

    with ExitStack() as stack:
        ec = stack.enter_context
        csem = [ec(nc.semaphore(f"c{j}")) for j in range(G + 1)]
        xbig = ec(nc.sbuf_tensor([P, (G + 1) * 2 * D], F32))
        dummies = ec(nc.sbuf_tensor([P, 8], F32))
        res = ec(nc.sbuf_tensor([P, 5 * NCOL + 5], F32))
        ha_sem = ec(nc.semaphore("ha_sem"))
        part_sem = ec(nc.semaphore("part_sem"))
        done_sem = ec(nc.semaphore("done_sem"))
        out_sem = ec(nc.semaphore("out_sem"))
        block = ec(nc.Block())
        # result columns interleaved per chunk: col = 5*j + quant_index,
        # then 5 spare columns (80..84) for chunk 0's B-half accumulators
        qidx = {q: i for i, q in enumerate(QUANTS)}
        xc = xbig.ap().rearrange("p (c v d) -> p c v d", v=2, d=D)

        def chunk(j, half, dslc=slice(None)):
            return xc[:, j, half, dslc]

        def rcol(q, j):
            k = 5 * j + qidx[q]
            return res.ap()[:, k:k + 1]

        def scol(k):
            return res.ap()[:, 5 * NCOL + k:5 * NCOL + k + 1]

        def bcast(k, n=D):
            return dummies.ap()[:, k:k + 1].broadcast_to((P, n))

        HA = slice(0, D // 2)
        HB = slice(D // 2, D)

        @block.sync
        def _(sync):
            # chunk 0/1 d-halves first, then chunk 2, the overlap chunk,
            # and the rest in order
            sync.dma_start(out=xc[:, 0, :, HA],
                           in_=x_v[0:P, :, 0, HA]).then_inc(ha_sem, 16)
            sync.dma_start(out=xc[:, 1, :, HA],
                           in_=x_v[0:P, :, 1, HA]).then_inc(ha_sem, 16)
            sync.dma_start(out=xc[:, 0, :, HB],
                           in_=x_v[0:P, :, 0, HB]).then_inc(csem[0], 16)
            sync.dma_start(out=xc[:, 1, :, HB],
                           in_=x_v[0:P, :, 1, HB]).then_inc(csem[1], 16)
            order = [2, G] + list(range(3, G))
            for j in order:
                src = x_v[0:P, :, j, :] if j < G else x_v[1:P + 1, :, 0, :]
                sync.dma_start(out=xc[:, j, :, :], in_=src).then_inc(
                    csem[j], 16)
            # cols 0:70 (chunks 0..13) as soon as both engines pass j=13
            sync.wait_ge(part_sem, 2)
            sync.dma_start(out=res_out[:, 0:70],
                           in_=res.ap()[:, 0:70]).then_inc(out_sem, 16)
            sync.wait_ge(done_sem, 2)
            sync.dma_start(out=res_out[:, 70:85],
                           in_=res.ap()[:, 70:85]).then_inc(out_sem, 16)
            sync.wait_ge(out_sem, 32)

        @block.scalar
        def _(scalar):
            # chunk 0 as d-halves (A accumulates into the j=0 column, B
            # into a spare column; host adds them)
            scalar.wait_ge(ha_sem, 16)
            scalar.activation(bcast(0, D // 2), chunk(0, 0, HA), AF.Square,
                              accum_out=rcol("ll", 0))
            scalar.activation(bcast(1, D // 2), chunk(0, 1, HA), AF.Square,
                              accum_out=rcol("tt", 0))
            scalar.wait_ge(csem[0], 16)
            scalar.activation(bcast(0, D // 2), chunk(0, 0, HB), AF.Square,
                              accum_out=scol(qidx["ll"]))
            scalar.activation(bcast(1, D // 2), chunk(0, 1, HB), AF.Square,
                              accum_out=scol(qidx["tt"]))
            scalar.wait_ge(ha_sem, 32)
            for j in range(1, G):
                scalar.wait_ge(csem[j], 16)
                scalar.activation(bcast(0), chunk(j, 0), AF.Square,
                                  accum_out=rcol("ll", j))
                ins = scalar.activation(bcast(1), chunk(j, 1), AF.Square,
                                        accum_out=rcol("tt", j))
                if j == 13:
                    ins.then_inc(part_sem, 1)
            ins.then_inc(done_sem, 1)

        @block.vector
        def _(vector):
            # chunk 0 (and its ln/tn partner chunk 1) as d-halves
            vector.wait_ge(ha_sem, 16)
            vector.scalar_tensor_tensor(
                out=bcast(2, D // 2), in0=chunk(0, 0, HA), scalar=0.0,
                in1=chunk(0, 1, HA), op0=ALU.bypass, op1=ALU.mult,
                accum_out=rcol("lt", 0))
            vector.wait_ge(ha_sem, 32)
            vector.scalar_tensor_tensor(
                out=bcast(3, D // 2), in0=chunk(0, 0, HA), scalar=0.0,
                in1=chunk(1, 0, HA), op0=ALU.bypass, op1=ALU.mult,
                accum_out=rcol("ln", 0))
            vector.scalar_tensor_tensor(
                out=bcast(4, D // 2), in0=chunk(0, 1, HA), scalar=0.0,
                in1=chunk(1, 1, HA), op0=ALU.bypass, op1=ALU.mult,
                accum_out=rcol("tn", 0))
            vector.wait_ge(csem[0], 16)
            vector.scalar_tensor_tensor(
                out=bcast(2, D // 2), in0=chunk(0, 0, HB), scalar=0.0,
                in1=chunk(0, 1, HB), op0=ALU.bypass, op1=ALU.mult,
                accum_out=scol(qidx["lt"]))
            vector.wait_ge(csem[1], 16)
            vector.scalar_tensor_tensor(
                out=bcast(3, D // 2), in0=chunk(0, 0, HB), scalar=0.0,
                in1=chunk(1, 0, HB), op0=ALU.bypass, op1=ALU.mult,
                accum_out=scol(qidx["ln"]))
            vector.scalar_tensor_tensor(
                out=bcast(4, D // 2), in0=chunk(0, 1, HB), scalar=0.0,
                in1=chunk(1, 1, HB), op0=ALU.bypass, op1=ALU.mult,
                accum_out=scol(qidx["tn"]))
            for j in range(1, G):
                vector.wait_ge(csem[j + 1], 16)
                vector.scalar_tensor_tensor(
                    out=bcast(2), in0=chunk(j, 0), scalar=0.0,
                    in1=chunk(j, 1), op0=ALU.bypass, op1=ALU.mult,
                    accum_out=rcol("lt", j))
                vector.scalar_tensor_tensor(
                    out=bcast(3), in0=chunk(j, 0), scalar=0.0,
                    in1=chunk(j + 1, 0), op0=ALU.bypass, op1=ALU.mult,
                    accum_out=rcol("ln", j))
                ins = vector.scalar_tensor_tensor(
                    out=bcast(4), in0=chunk(j, 1), scalar=0.0,
                    in1=chunk(j + 1, 1), op0=ALU.bypass, op1=ALU.mult,
                    accum_out=rcol("tn", j))
                if j == 13:
                    ins.then_inc(part_sem, 1)
            ins.then_inc(done_sem, 1)

    _cached_nc = nc
    return nc


def _unpack(arr):
    """(128, NCOL) device layout -> (B_SHARD, S): row r = p*G + j."""
    return arr.reshape(ROWS).reshape(B_SHARD, S)


def _run_device(logits, tgt_out, trace=False):
    """Returns dict q -> (B, S) float32 row-dot arrays, plus kernel results."""
    nc = _build_program()
    # (S, B, D) -> (B, S, D) batch-major, split over cores along B
    lb = np.ascontiguousarray(np.swapaxes(logits, 0, 1))
    tb = np.ascontiguousarray(np.swapaxes(tgt_out, 0, 1))
    in_maps = []
    for c in range(N_CORES):
        sl = slice(c * B_SHARD, (c + 1) * B_SHARD)
        x = np.zeros((2, ROWS_PAD, D), np.float32)
        x[0, :ROWS] = lb[sl].reshape(ROWS, D)
        x[1, :ROWS] = tb[sl].reshape(ROWS, D)
        in_maps.append({"x": x})
    kres = run_bass_kernel_spmd(nc, in_maps, list(range(N_CORES)), trace=trace)
    full = {}
    for i, q in enumerate(QUANTS):
        parts = []
        for c in range(N_CORES):
            r = kres.results[c]["res"]
            arr = r[:, :5 * NCOL].reshape(P, NCOL, 5)[:, :, i].copy()
            arr[:, 0] += r[:, 5 * NCOL + i]      # chunk 0's B-half
            parts.append(_unpack(arr))
        full[q] = np.concatenate(parts, axis=0)
    return full, kres


def _finish_host(rows, mask):
    """Host-side float64 finish: reproduce reference semantics exactly."""
    ll = rows["ll"].astype(np.float64)
    tt = rows["tt"].astype(np.float64)
    lt = rows["lt"].astype(np.float64)
    ln = rows["ln"].astype(np.float64)
    tn = rows["tn"].astype(np.float64)

    valid = ~mask                     # (B, S)
    n_valid = float(valid.sum())

    # masked MSE: sum over valid rows of sum_d (l-t)^2 = ll - 2lt + tt
    mse = ((ll - 2.0 * lt + tt) * valid).sum() / (n_valid * D)

    # CosineEmbeddingLoss part (eps = 1e-8)
    na = np.maximum(np.sqrt(ll), 1e-8)
    nb = np.maximum(np.sqrt(tt), 1e-8)
    c = lt / (na * nb)
    loss_cos = ((1.0 - c) * valid).sum() / n_valid

    # consecutive-sentence cosine deltas (eps = 1e-6), shape (B, S-1)
    nl = np.maximum(np.sqrt(ll), 1e-6)
    nt = np.maximum(np.sqrt(tt), 1e-6)
    d_l = ln[:, :S - 1] / (nl[:, :-1] * nl[:, 1:])
    d_t = tn[:, :S - 1] / (nt[:, :-1] * nt[:, 1:])
    pair_valid = valid[:, :-1] & valid[:, 1:]
    cnt = int(pair_valid.sum())
    loss_delta = (np.square(d_l - d_t) * pair_valid).sum() / max(cnt, 1)

    # delta-of-delta on the compacted (valid-only, batch-major) delta lists
    L = B * (S - 1)
    pvf = pair_valid.reshape(-1)

    def dd(d_flat):
        dense = np.zeros(L, np.float64)
        dense[:cnt] = d_flat[pvf]
        prev = dense[:-1]
        den = np.where(prev != 0, prev, 1e-6)
        return (dense[1:] - prev) / den

    dd_l = dd(d_l.reshape(-1))
    dd_t = dd(d_t.reshape(-1))
    dd_valid = np.arange(L - 1) < (cnt - 1)
    n_dd = float(max(cnt - 1, 1))
    loss_dd = (np.square(dd_l - dd_t) * dd_valid).sum() / n_dd / 100.0

    return mse + loss_cos + loss_delta + loss_dd


def kernel(logits, tgt_out, tgt_padding_mask, _trace=False):
    logits = np.asarray(logits, dtype=np.float32)
    tgt_out = np.asarray(tgt_out, dtype=np.float32)
    mask = np.asarray(tgt_padding_mask).astype(bool)
    rows, kres = _run_device(logits, tgt_out, trace=_trace)
    total = _finish_host(rows, mask)
    out = np.array(total, dtype=np.float32)
    if _trace:
        return out, kres
    return out
